# revision 54
# baseline (speedup 1.0000x reference)
"""Trainium2 Bass kernel for the dense branch-MLP problem (fp8 DoubleRow).

Computes: out[b,o] = sum_n relu((s[b,:] - v[n,:]) @ W[n].T + bias[n])[o]
with B=1024, N=64, D=512, OUT=2048 in fp32, graded at rel_absmax < 2e-2.

Sharding: expert-style across the N=64 branch axis -> 8 branches per core.
Each core computes a full [B, OUT] partial sum over its 8 branches; the
host sums the 8 partials and descales (the unshard step).

Math restructure (host side):
  (s - v_n) @ W_n^T + b_n  ==  s @ W_n^T + c_n,   c_n = b_n - v_n @ W_n^T
Both s and (aw*W) are quantized to fp8 e4m3 on the host (aw=16 keeps W out
of the subnormal range); full-chain emulated rel_absmax ~= 1.35e-2.

FLIPPED-LAYOUT schedule: psum partitions = batch, free = output. Each unit
(nl, bt, h) = one branch x one 128-batch tile x one 1024-wide output half
in a [128, 1024] psum span (2 banks, 4-deep ring):
  - per 512-wide psum bank: one partition-1 fp8 DoubleRow "bias" matmul
    (ones stationary; c_n hi/lo fp8 rows moving, parked per output chunk
    at partition 0/32/64 x free slot) pre-loads c_n into psum, then 2
    DoubleRow matmuls accumulate s @ W_n^T on top,
  - the drain is then BIAS-FREE, so DVE fuses relu+accumulate in one
    scalar_tensor_tensor pass (acc[bt] = (psum MAX 0) ADD acc[bt]) and ACT
    does plain relus into tmp, added into acc[bt] via DVE tensor_tensor or
    gpsimd accumulating DMAs,
  - each acc[bt] half-chain has one link per branch, ~11us apart: add
    latency is invisible. Out DMA per output half right after branch 7's
    link so the serial DMA pipeline never backlogs.
PE is the bottleneck (512 branch + 256 bias DoubleRow matmuls, ~88us) and
runs near-continuously, keeping the clock-ramp p-state at full speed.
Cost-model timeline: 98.6us/core (was 235.6us for the fp32r baseline);
measured end-to-end rel_absmax 1.354e-2.
"""

import numpy as np
import ml_dtypes

import concourse.bacc as bacc
import concourse.mybir as mybir
import concourse.tile as tile
from concourse.bass_utils import run_bass_kernel_spmd

B, N, D, OUT = 1024, 64, 512, 2048
N_CORES = 8
NL = N // N_CORES  # branches per core (8)
DC = D // 128      # contraction k-tiles (4)
BT = B // 128      # batch tiles (8)
NQ = 4             # weight-stream quads
AW = 16.0          # host-side weight scale before fp8 quantization

F32 = mybir.dt.float32
BF16 = mybir.dt.bfloat16
F8 = mybir.dt.float8e4
RELU = mybir.ActivationFunctionType.Relu
DR = mybir.MatmulPerfMode.DoubleRow
ADD = mybir.AluOpType.add
MAX = mybir.AluOpType.max

# -- static schedules -------------------------------------------------------
# Unit (nl, bt, h) drain: 'D' = DVE fused stt, 'A' = ACT relu (+add, nl>0).
N_D_UNITS = 54


def _spread(total, picks):
    return [(i * picks) // total != ((i - 1) * picks) // total for i in range(total)]


_DMASK = _spread(128, N_D_UNITS)
_FORM = {}
for _nl in range(NL):
    for _bt in range(BT):
        for _h in range(2):
            _FORM[(_nl, _bt, _h)] = "D" if _DMASK[_nl * 16 + _bt * 2 + _h] else "A"
_FORM[(7, 6, 1)] = _FORM[(7, 7, 0)] = _FORM[(7, 7, 1)] = "A"


def _a_add_path(nl, bt, k):
    """Path for an A-unit's add into acc[bt] ('V'/'M'/'P'). k spreads it."""
    if nl <= 1:
        return "P" if bt % 2 else "V"
    if nl >= NL - 1:
        return "V"
    return "M"


_cache = {}


def build(repeat: int = 1):
    """Build + compile the per-core Bass program. Cached per `repeat`."""
    if repeat in _cache:
        return _cache[repeat]

    nc = bacc.Bacc(
        "TRN2",
        target_bir_lowering=False,
        debug=False,
        num_devices=N_CORES,
    )

    # weights pre-chunked on host: [nl, quad, 128, DC*512] fp8
    wt_d = nc.dram_tensor("wt", [NL, NQ, 128, DC * 512], F8, kind="ExternalInput").ap()
    st_d = nc.dram_tensor("st", [128, DC * B], F8, kind="ExternalInput").ap()
    # bias rows: partition p(oc) in {0,32,64}, slot s(oc): [nl, hi/lo, 512]
    c8_d = nc.dram_tensor("c8", [128, NL * 2 * 2 * 512], F8, kind="ExternalInput").ap()
    ones_d = nc.dram_tensor("ones", [128, 256], F8, kind="ExternalInput").ap()
    out_d = nc.dram_tensor("out", [128, BT * OUT], BF16, kind="ExternalOutput").ap()

    with tile.TileContext(nc) as tc:
        with (
            tc.tile_pool(name="const", bufs=1) as const_pool,
            tc.tile_pool(name="acc", bufs=1) as acc_pool,
            tc.tile_pool(name="tmp", bufs=4) as tmp_pool,
            tc.tile_pool(name="wt", bufs=1) as wt_pool,
            tc.tile_pool(name="psum", bufs=4, space="PSUM") as psum_pool,
        ):
            ones = const_pool.tile([128, 256], F8, name="ones")
            nc.sync.dma_start(ones[:], ones_d[:])
            c8 = const_pool.tile([128, NL, 2, 2, 512], F8, name="c8")
            c8_d5 = c8_d.rearrange("p (n k s o) -> p n k s o", n=NL, k=2, s=2)

            def c8_chunk_dma(nl):
                nc.sync.dma_start(c8[:, nl], c8_d5[:, nl])

            st = const_pool.tile([128, DC, B], F8, name="st")
            st_d3 = st_d.rearrange("p (c b) -> p c b", c=DC)

            # one resident weight tile per branch, filled quad-by-quad
            wts = [
                wt_pool.tile([128, DC, OUT], F8, name=f"wt{nl}", tag=f"wt{nl}")
                for nl in range(NL)
            ]

            def wt_chunk_dma(nl, q):
                wd3 = wt_d[nl, q].rearrange("p (c o) -> p c o", c=DC)
                nc.sync.dma_start(wts[nl][:, :, q * 512 : q * 512 + 512], wd3)

            c8_chunk_dma(0)
            for c in range(DC):
                nc.sync.dma_start(st[:, c], st_d3[:, c])
            wt_chunk_dma(0, 0)
            wt_chunk_dma(0, 1)
            c8_chunk_dma(1)
            wt_chunk_dma(0, 2)
            wt_chunk_dma(0, 3)

            # PE p-state warmup burst during the startup DMA window.
            scr = const_pool.tile([128, 128], BF16, name="scr")
            nc.vector.memset(scr[:], 0.0)
            wps = psum_pool.tile([128, 1024], F32, name="ps", tag="ps")
            for _ in range(56):
                nc.tensor.matmul(
                    wps[0:64, 0:64], scr[:, 0:64], scr[:, 64:128], start=True, stop=True
                )

            # per-bt bf16 accumulators, resident across all branches
            accs = [
                acc_pool.tile([128, OUT], BF16, name=f"acc{bt}", tag=f"acc{bt}")
                for bt in range(BT)
            ]

            def add_op(path, dst, src):
                if path == "V":
                    nc.vector.tensor_add(dst, dst, src)
                elif path == "M":
                    nc.gpsimd.dma_start(dst, src, accum_op=ADD)
                else:
                    nc.gpsimd.tensor_add(dst, dst, src)

            def body(iv=None):
                a_pend = []  # (nl, bt, h, tmp, k) adds awaiting emission
                a_cnt = 0

                def flush_one():
                    anl, abt, ah, at, k = a_pend.pop(0)
                    dst = accs[abt][:, ah * 1024 : ah * 1024 + 1024]
                    add_op(_a_add_path(anl, abt, k), dst, at[:])

                for nl in range(NL):
                    for bt in range(BT):
                        for h in range(2):
                            ps = psum_pool.tile([128, 1024], F32, name="ps", tag="ps")
                            wt = wts[nl]
                            for j in range(2):
                                oc = 2 * h + j
                                osl = slice(oc * 512, oc * 512 + 512)
                                psl = slice(j * 512, j * 512 + 512)
                                p0, slot = ((0, 0), (32, 0), (64, 0), (0, 1))[oc]
                                ones3 = ones[p0 : p0 + 1, :].rearrange(
                                    "p (k f) -> p k f", k=2
                                )
                                nc.tensor.matmul(
                                    ps[:, psl],
                                    ones3,
                                    c8[p0 : p0 + 1, nl, :, slot, :],
                                    start=True,
                                    stop=False,
                                    perf_mode=DR,
                                )
                                for ci in range(2):
                                    nc.tensor.matmul(
                                        ps[:, psl],
                                        st[:, 2 * ci : 2 * ci + 2, bt * 128 : bt * 128 + 128],
                                        wt[:, 2 * ci : 2 * ci + 2, osl],
                                        start=False,
                                        stop=(ci == 1),
                                        perf_mode=DR,
                                    )
                            acc_h = accs[bt][:, h * 1024 : h * 1024 + 1024]
                            if _FORM[(nl, bt, h)] == "D":
                                if nl == 0:
                                    nc.vector.tensor_scalar(
                                        acc_h, ps[:], 0.0, None, op0=MAX
                                    )
                                else:
                                    nc.vector.scalar_tensor_tensor(
                                        acc_h, ps[:], 0.0, acc_h, op0=MAX, op1=ADD
                                    )
                            else:
                                if nl == 0:
                                    nc.scalar.activation(
                                        acc_h, ps[:], RELU, bias=0.0, scale=1.0
                                    )
                                else:
                                    t = tmp_pool.tile(
                                        [128, 1024], BF16, name="tmp", tag=f"tmp{(bt * 2 + h) % 4}"
                                    )
                                    nc.scalar.activation(
                                        t[:], ps[:], RELU, bias=0.0, scale=1.0
                                    )
                                    a_pend.append((nl, bt, h, t, a_cnt))
                                    a_cnt += 1
                            # lagged A-adds (~2 units behind)
                            if len(a_pend) >= 3:
                                flush_one()
                            # out DMA per half as branch 7's link completes
                            if nl == NL - 1:
                                for item in [x for x in a_pend if x[1] == bt and x[2] == h]:
                                    a_pend.remove(item)
                                    dst = accs[bt][:, item[2] * 1024 : item[2] * 1024 + 1024]
                                    add_op(_a_add_path(item[0], bt, item[4]), dst, item[3][:])
                                nc.sync.dma_start(
                                    out_d[:, bt * OUT + h * 1024 : bt * OUT + h * 1024 + 1024],
                                    accs[bt][:, h * 1024 : h * 1024 + 1024],
                                )
                        # weight + bias prefetch for the next branch
                        if nl < NL - 1 and bt in (1, 3, 5, 7):
                            wt_chunk_dma(nl + 1, (bt - 1) // 2)
                            if bt == 1 and nl < NL - 2:
                                c8_chunk_dma(nl + 2)
                for item in a_pend:
                    dst = accs[item[1]][:, item[2] * 1024 : item[2] * 1024 + 1024]
                    add_op(_a_add_path(item[0], item[1], item[4]), dst, item[3][:])

            if repeat == 1:
                body()
            else:
                with tc.For_i(0, repeat, 1):
                    body()

    nc.compile()
    _cache[repeat] = nc
    return nc


def prep_inputs(semantic_vec, vertices, W, b):
    """Host-side layout transforms + fp8 quantization -> per-core inputs."""
    s64 = np.asarray(semantic_vec, dtype=np.float64)
    v64 = np.asarray(vertices, dtype=np.float64)
    W64 = np.asarray(W, dtype=np.float64)
    b64 = np.asarray(b, dtype=np.float64)
    f8 = ml_dtypes.float8_e4m3fn

    # c[n, o] = b[n, o] - v[n] @ W[n].T  (exact, f64), AW-scaled, fp8 hi/lo
    c = AW * (b64 - np.einsum("nd,nod->no", v64, W64))
    chi = c.astype(f8)
    clo = (c - chi.astype(np.float64)).astype(f8)
    # c8[core][p(oc), ((nl*2 + k)*2 + s(oc))*512 + j] = (hi,lo)[k][n, oc*512+j]
    c8 = np.zeros((N_CORES, 128, NL * 2 * 2 * 512), dtype=f8)
    _OCMAP = ((0, 0), (32, 0), (64, 0), (0, 1))
    for nl in range(NL):
        for k, arr in enumerate((chi, clo)):
            a4 = arr.reshape(N_CORES, NL, 4, 512)
            for oc in range(4):
                p0, s = _OCMAP[oc]
                base = ((nl * 2 + k) * 2 + s) * 512
                c8[:, p0, base : base + 512] = a4[:, nl, oc, :]

    # st8[p, c*B + bb] = fp8(s[bb, c*128+p])
    st8 = np.ascontiguousarray(
        s64.reshape(B, DC, 128).transpose(2, 1, 0).reshape(128, DC * B)
    ).astype(f8)
    # wt8[n, q, p, c*512 + oo] = fp8(AW * W[n, q*512 + oo, c*128+p])
    wt8 = np.ascontiguousarray(
        (AW * W64)
        .reshape(N, NQ, 512, DC, 128)      # [n, q, oo, c, p]
        .transpose(0, 1, 4, 3, 2)          # [n, q, p, c, oo]
        .reshape(N, NQ, 128, DC * 512)
    ).astype(f8)
    ones = np.ones((128, 256), dtype=f8)

    in_maps = []
    for core in range(N_CORES):
        in_maps.append(
            {
                "wt": wt8[core * NL : (core + 1) * NL],
                "st": st8,
                "c8": c8[core],
                "ones": ones,
            }
        )
    return in_maps


def kernel(semantic_vec, vertices, W, b):
    nc = build(repeat=1)
    in_maps = prep_inputs(semantic_vec, vertices, W, b)
    res = run_bass_kernel_spmd(nc, in_maps, core_ids=list(range(N_CORES)))
    total = np.zeros((B, OUT), dtype=np.float32)
    for core in range(N_CORES):
        o = np.asarray(res.results[core]["out"]).astype(np.float32)
        # o[p, bt*OUT + oo] -> out[bt*128 + p, oo]
        total += o.reshape(128, BT, OUT).transpose(1, 0, 2).reshape(B, OUT)
    total *= np.float32(1.0 / AW)
    return np.ascontiguousarray(total)


# revision 56
# speedup vs baseline: 1.0004x; 1.0004x over previous
"""Trainium2 Bass kernel for the dense branch-MLP problem (fp8 DoubleRow).

Computes: out[b,o] = sum_n relu((s[b,:] - v[n,:]) @ W[n].T + bias[n])[o]
with B=1024, N=64, D=512, OUT=2048 in fp32, graded at rel_absmax < 2e-2.

Sharding: expert-style across the N=64 branch axis -> 8 branches per core.
Each core computes a full [B, OUT] partial sum over its 8 branches; the
host sums the 8 partials and descales (the unshard step).

Math restructure (host side):
  (s - v_n) @ W_n^T + b_n  ==  s @ W_n^T + c_n,   c_n = b_n - v_n @ W_n^T
Both s and (aw*W) are quantized to fp8 e4m3 on the host (aw=16 keeps W out
of the subnormal range); full-chain emulated rel_absmax ~= 1.35e-2.

FLIPPED-LAYOUT schedule: psum partitions = batch, free = output. Each unit
(nl, bt, h) = one branch x one 128-batch tile x one 1024-wide output half
in a [128, 1024] psum span (2 banks, 4-deep ring):
  - per 512-wide psum bank: one partition-1 fp8 DoubleRow "bias" matmul
    (ones stationary; c_n hi/lo fp8 rows moving, parked per output chunk
    at partition 0/32/64 x free slot) pre-loads c_n into psum, then 2
    DoubleRow matmuls accumulate s @ W_n^T on top,
  - the drain is then BIAS-FREE, so DVE fuses relu+accumulate in one
    scalar_tensor_tensor pass (acc[bt] = (psum MAX 0) ADD acc[bt]) and ACT
    does plain relus into tmp, added into acc[bt] via DVE tensor_tensor or
    gpsimd accumulating DMAs,
  - each acc[bt] half-chain has one link per branch, ~11us apart: add
    latency is invisible. Out DMA per output half right after branch 7's
    link so the serial DMA pipeline never backlogs.
PE is the bottleneck (512 branch + 256 bias DoubleRow matmuls, ~88us) and
runs near-continuously, keeping the clock-ramp p-state at full speed.
Cost-model timeline: 98.58us/core (was 235.6us for the fp32r baseline);
measured end-to-end rel_absmax 1.296e-2 on hardware.
"""

import numpy as np
import ml_dtypes

import concourse.bacc as bacc
import concourse.mybir as mybir
import concourse.tile as tile
from concourse.bass_utils import run_bass_kernel_spmd

B, N, D, OUT = 1024, 64, 512, 2048
N_CORES = 8
NL = N // N_CORES  # branches per core (8)
DC = D // 128      # contraction k-tiles (4)
BT = B // 128      # batch tiles (8)
NQ = 4             # weight-stream quads
AW = 16.0          # host-side weight scale before fp8 quantization

F32 = mybir.dt.float32
BF16 = mybir.dt.bfloat16
F8 = mybir.dt.float8e4
RELU = mybir.ActivationFunctionType.Relu
DR = mybir.MatmulPerfMode.DoubleRow
ADD = mybir.AluOpType.add
MAX = mybir.AluOpType.max

# -- static schedules -------------------------------------------------------
# Unit (nl, bt, h) drain: 'D' = DVE fused stt, 'A' = ACT relu (+add, nl>0).
N_D_UNITS = 53


def _spread(total, picks):
    return [(i * picks) // total != ((i - 1) * picks) // total for i in range(total)]


_DMASK = _spread(128, N_D_UNITS)
_FORM = {}
for _nl in range(NL):
    for _bt in range(BT):
        for _h in range(2):
            _FORM[(_nl, _bt, _h)] = "D" if _DMASK[_nl * 16 + _bt * 2 + _h] else "A"
_FORM[(7, 6, 1)] = _FORM[(7, 7, 0)] = _FORM[(7, 7, 1)] = "A"


def _a_add_path(nl, bt, k):
    """Path for an A-unit's add into acc[bt] ('V'/'M'/'P'). k spreads it."""
    if nl <= 1:
        return "P" if bt % 2 else "V"
    if nl >= NL - 1:
        return "V"
    return "M"


_cache = {}


def build(repeat: int = 1):
    """Build + compile the per-core Bass program. Cached per `repeat`."""
    if repeat in _cache:
        return _cache[repeat]

    nc = bacc.Bacc(
        "TRN2",
        target_bir_lowering=False,
        debug=False,
        num_devices=N_CORES,
    )

    # weights pre-chunked on host: [nl, quad, 128, DC*512] fp8
    wt_d = nc.dram_tensor("wt", [NL, NQ, 128, DC * 512], F8, kind="ExternalInput").ap()
    st_d = nc.dram_tensor("st", [128, DC * B], F8, kind="ExternalInput").ap()
    # bias rows: partition p(oc) in {0,32,64}, slot s(oc): [nl, hi/lo, 512]
    c8_d = nc.dram_tensor("c8", [128, NL * 2 * 2 * 512], F8, kind="ExternalInput").ap()
    ones_d = nc.dram_tensor("ones", [128, 256], F8, kind="ExternalInput").ap()
    out_d = nc.dram_tensor("out", [128, BT * OUT], BF16, kind="ExternalOutput").ap()

    with tile.TileContext(nc) as tc:
        with (
            tc.tile_pool(name="const", bufs=1) as const_pool,
            tc.tile_pool(name="acc", bufs=1) as acc_pool,
            tc.tile_pool(name="tmp", bufs=4) as tmp_pool,
            tc.tile_pool(name="wt", bufs=1) as wt_pool,
            tc.tile_pool(name="psum", bufs=4, space="PSUM") as psum_pool,
        ):
            ones = const_pool.tile([128, 256], F8, name="ones")
            nc.sync.dma_start(ones[:], ones_d[:])
            c8 = const_pool.tile([128, NL, 2, 2, 512], F8, name="c8")
            c8_d5 = c8_d.rearrange("p (n k s o) -> p n k s o", n=NL, k=2, s=2)

            def c8_chunk_dma(nl):
                nc.sync.dma_start(c8[:, nl], c8_d5[:, nl])

            st = const_pool.tile([128, DC, B], F8, name="st")
            st_d3 = st_d.rearrange("p (c b) -> p c b", c=DC)

            # one resident weight tile per branch, filled quad-by-quad
            wts = [
                wt_pool.tile([128, DC, OUT], F8, name=f"wt{nl}", tag=f"wt{nl}")
                for nl in range(NL)
            ]

            def wt_chunk_dma(nl, q):
                wd3 = wt_d[nl, q].rearrange("p (c o) -> p c o", c=DC)
                nc.sync.dma_start(wts[nl][:, :, q * 512 : q * 512 + 512], wd3)

            c8_chunk_dma(0)
            for c in range(DC):
                nc.sync.dma_start(st[:, c], st_d3[:, c])
            wt_chunk_dma(0, 0)
            wt_chunk_dma(0, 1)
            c8_chunk_dma(1)
            wt_chunk_dma(0, 2)
            wt_chunk_dma(0, 3)

            # PE p-state warmup burst during the startup DMA window.
            scr = const_pool.tile([128, 128], BF16, name="scr")
            nc.vector.memset(scr[:], 0.0)
            wps = psum_pool.tile([128, 1024], F32, name="ps", tag="ps")
            for _ in range(56):
                nc.tensor.matmul(
                    wps[0:64, 0:64], scr[:, 0:64], scr[:, 64:128], start=True, stop=True
                )

            # per-bt bf16 accumulators, resident across all branches
            accs = [
                acc_pool.tile([128, OUT], BF16, name=f"acc{bt}", tag=f"acc{bt}")
                for bt in range(BT)
            ]

            def add_op(path, dst, src):
                if path == "V":
                    nc.vector.tensor_add(dst, dst, src)
                elif path == "M":
                    nc.gpsimd.dma_start(dst, src, accum_op=ADD)
                else:
                    nc.gpsimd.tensor_add(dst, dst, src)

            def body(iv=None):
                a_pend = []  # (nl, bt, h, tmp, k) adds awaiting emission
                a_cnt = 0

                def flush_one():
                    anl, abt, ah, at, k = a_pend.pop(0)
                    dst = accs[abt][:, ah * 1024 : ah * 1024 + 1024]
                    add_op(_a_add_path(anl, abt, k), dst, at[:])

                for nl in range(NL):
                    for bt in range(BT):
                        for h in range(2):
                            ps = psum_pool.tile([128, 1024], F32, name="ps", tag="ps")
                            wt = wts[nl]
                            for j in range(2):
                                oc = 2 * h + j
                                osl = slice(oc * 512, oc * 512 + 512)
                                psl = slice(j * 512, j * 512 + 512)
                                p0, slot = ((0, 0), (32, 0), (64, 0), (0, 1))[oc]
                                ones3 = ones[p0 : p0 + 1, :].rearrange(
                                    "p (k f) -> p k f", k=2
                                )
                                nc.tensor.matmul(
                                    ps[:, psl],
                                    ones3,
                                    c8[p0 : p0 + 1, nl, :, slot, :],
                                    start=True,
                                    stop=False,
                                    perf_mode=DR,
                                )
                                for ci in range(2):
                                    nc.tensor.matmul(
                                        ps[:, psl],
                                        st[:, 2 * ci : 2 * ci + 2, bt * 128 : bt * 128 + 128],
                                        wt[:, 2 * ci : 2 * ci + 2, osl],
                                        start=False,
                                        stop=(ci == 1),
                                        perf_mode=DR,
                                    )
                            acc_h = accs[bt][:, h * 1024 : h * 1024 + 1024]
                            if _FORM[(nl, bt, h)] == "D":
                                if nl == 0:
                                    nc.vector.tensor_scalar(
                                        acc_h, ps[:], 0.0, None, op0=MAX
                                    )
                                else:
                                    nc.vector.scalar_tensor_tensor(
                                        acc_h, ps[:], 0.0, acc_h, op0=MAX, op1=ADD
                                    )
                            else:
                                if nl == 0:
                                    nc.scalar.activation(
                                        acc_h, ps[:], RELU, bias=0.0, scale=1.0
                                    )
                                else:
                                    t = tmp_pool.tile(
                                        [128, 1024], BF16, name="tmp", tag=f"tmp{(bt * 2 + h) % 4}"
                                    )
                                    nc.scalar.activation(
                                        t[:], ps[:], RELU, bias=0.0, scale=1.0
                                    )
                                    a_pend.append((nl, bt, h, t, a_cnt))
                                    a_cnt += 1
                            # lagged A-adds (~2 units behind)
                            if len(a_pend) >= 3:
                                flush_one()
                            # out DMA per half as branch 7's link completes
                            if nl == NL - 1:
                                for item in [x for x in a_pend if x[1] == bt and x[2] == h]:
                                    a_pend.remove(item)
                                    dst = accs[bt][:, item[2] * 1024 : item[2] * 1024 + 1024]
                                    add_op(_a_add_path(item[0], bt, item[4]), dst, item[3][:])
                                nc.sync.dma_start(
                                    out_d[:, bt * OUT + h * 1024 : bt * OUT + h * 1024 + 1024],
                                    accs[bt][:, h * 1024 : h * 1024 + 1024],
                                )
                        # weight + bias prefetch for the next branch
                        if nl < NL - 1 and bt in (1, 3, 5, 7):
                            wt_chunk_dma(nl + 1, (bt - 1) // 2)
                            if bt == 1 and nl < NL - 2:
                                c8_chunk_dma(nl + 2)
                for item in a_pend:
                    dst = accs[item[1]][:, item[2] * 1024 : item[2] * 1024 + 1024]
                    add_op(_a_add_path(item[0], item[1], item[4]), dst, item[3][:])

            if repeat == 1:
                body()
            else:
                with tc.For_i(0, repeat, 1):
                    body()

    nc.compile()
    _cache[repeat] = nc
    return nc


def prep_inputs(semantic_vec, vertices, W, b):
    """Host-side layout transforms + fp8 quantization -> per-core inputs."""
    s64 = np.asarray(semantic_vec, dtype=np.float64)
    v64 = np.asarray(vertices, dtype=np.float64)
    W64 = np.asarray(W, dtype=np.float64)
    b64 = np.asarray(b, dtype=np.float64)
    f8 = ml_dtypes.float8_e4m3fn

    # c[n, o] = b[n, o] - v[n] @ W[n].T  (exact, f64), AW-scaled, fp8 hi/lo
    c = AW * (b64 - np.einsum("nd,nod->no", v64, W64))
    chi = c.astype(f8)
    clo = (c - chi.astype(np.float64)).astype(f8)
    # c8[core][p(oc), ((nl*2 + k)*2 + s(oc))*512 + j] = (hi,lo)[k][n, oc*512+j]
    c8 = np.zeros((N_CORES, 128, NL * 2 * 2 * 512), dtype=f8)
    _OCMAP = ((0, 0), (32, 0), (64, 0), (0, 1))
    for nl in range(NL):
        for k, arr in enumerate((chi, clo)):
            a4 = arr.reshape(N_CORES, NL, 4, 512)
            for oc in range(4):
                p0, s = _OCMAP[oc]
                base = ((nl * 2 + k) * 2 + s) * 512
                c8[:, p0, base : base + 512] = a4[:, nl, oc, :]

    # st8[p, c*B + bb] = fp8(s[bb, c*128+p])
    st8 = np.ascontiguousarray(
        s64.reshape(B, DC, 128).transpose(2, 1, 0).reshape(128, DC * B)
    ).astype(f8)
    # wt8[n, q, p, c*512 + oo] = fp8(AW * W[n, q*512 + oo, c*128+p])
    wt8 = np.ascontiguousarray(
        (AW * W64)
        .reshape(N, NQ, 512, DC, 128)      # [n, q, oo, c, p]
        .transpose(0, 1, 4, 3, 2)          # [n, q, p, c, oo]
        .reshape(N, NQ, 128, DC * 512)
    ).astype(f8)
    ones = np.ones((128, 256), dtype=f8)

    in_maps = []
    for core in range(N_CORES):
        in_maps.append(
            {
                "wt": wt8[core * NL : (core + 1) * NL],
                "st": st8,
                "c8": c8[core],
                "ones": ones,
            }
        )
    return in_maps


def kernel(semantic_vec, vertices, W, b):
    nc = build(repeat=1)
    in_maps = prep_inputs(semantic_vec, vertices, W, b)
    res = run_bass_kernel_spmd(nc, in_maps, core_ids=list(range(N_CORES)))
    total = np.zeros((B, OUT), dtype=np.float32)
    for core in range(N_CORES):
        o = np.asarray(res.results[core]["out"]).astype(np.float32)
        # o[p, bt*OUT + oo] -> out[bt*128 + p, oo]
        total += o.reshape(128, BT, OUT).transpose(1, 0, 2).reshape(B, OUT)
    total *= np.float32(1.0 / AW)
    return np.ascontiguousarray(total)
